# revision 3
# baseline (speedup 1.0000x reference)
"""nn_CDA Trainium kernel: fp16 dst-sharded GCN message passing on 8 NeuronCores.

Per 3-layer GCN stack (one bass program per graph family, SPMD on 8 cores):
  phase 1: xt blocks via HWDGE transpose-DMA, h' = dinv*(x@W) on PE,
           ACT scale+cast f16 into wide tiles, grouped stores.
  AllGather h' (f16).
  phase 2: idx/dstloc/normv tables SBUF-resident (loaded once per launch).
           Per 2-block group: lo/hi dma_gather (f16 256B rows) round-robin
           over 4 SWDGE queues (4x gather throughput vs 1 queue);
           per 128-edge chunk: DVE one-hot (iota==dstloc)*norm -> PE scatter
           matmul accumulating in PSUM; rank-1 bias matmul; ReLU*dinv on ACT.
Host: degree/norm preprocessing, PCA (CPU eigh like the reference), decoder.
"""
import numpy as np

P = 128
N_CORES = 8
LO = 32768
FC = 128
K = 64
TRW = 4   # node blocks per transpose-DMA load
GB = 2    # dst blocks per gather group
NC, ND, PPAIR = 60000, 20000, 200000
NPAD_C = 60416   # 8*59*128
NPAD_D = 20480   # 8*20*128


def _cdiv(a, b):
    return -(-a // b)


def _prep_counts(src, dst, ew, n_real, npad):
    sh = npad // N_CORES
    nblk = sh // P
    src = np.concatenate([np.asarray(src, np.int64), np.arange(n_real, dtype=np.int64)])
    dst = np.concatenate([np.asarray(dst, np.int64), np.arange(n_real, dtype=np.int64)])
    ew = np.concatenate([np.asarray(ew, np.float32), np.ones(n_real, np.float32)])
    gblk = dst >> 7
    hi = (src >= LO).astype(np.int64)
    nkeys = N_CORES * nblk * 2
    key = gblk * 2 + hi
    cnt = np.bincount(key, minlength=nkeys)
    cnt_lo = cnt[0::2].reshape(N_CORES, nblk)
    cnt_hi = cnt[1::2].reshape(N_CORES, nblk)
    nlo = _cdiv(cnt_lo, P).max(axis=0)
    nhi = _cdiv(cnt_hi, P).max(axis=0)
    return dict(src=src, dst=dst, ew=ew, gblk=gblk, hi=hi, key=key, cnt=cnt,
                nlo=nlo, nhi=nhi, sh=sh, nblk=nblk)


def _group_layout(nlo, nhi):
    nblk = len(nlo)
    ngrp = _cdiv(nblk, GB)
    glo = np.zeros(ngrp, np.int64)
    ghi = np.zeros(ngrp, np.int64)
    lo_col = np.zeros(nblk, np.int64)
    hi_col = np.zeros(nblk, np.int64)
    gstart = np.zeros(ngrp, np.int64)
    col = 0
    for gi in range(ngrp):
        bs = range(gi * GB, min((gi + 1) * GB, nblk))
        gstart[gi] = col
        glo[gi] = sum(int(nlo[b]) for b in bs)
        ghi[gi] = sum(int(nhi[b]) for b in bs)
        c = col
        for b in bs:
            lo_col[b] = c
            c += int(nlo[b])
        for b in bs:
            hi_col[b] = c
            c += int(nhi[b])
        col += int(glo[gi] + ghi[gi])
    return dict(ngrp=ngrp, glo=glo, ghi=ghi, lo_col=lo_col, hi_col=hi_col,
                gstart=gstart, totch=int(col))


def _prep_pack(c, npad, nlo, nhi):
    sh, nblk = c["sh"], c["nblk"]
    src, dst, ew = c["src"], c["dst"], c["ew"]
    E = src.shape[0]

    deg = np.bincount(dst, weights=ew.astype(np.float64), minlength=npad)
    nz = deg > 0
    dinv = np.zeros(npad, np.float32)
    dinv[nz] = (1.0 / np.sqrt(deg[nz])).astype(np.float32)
    recip = np.zeros(npad, np.float32)
    recip[nz] = np.sqrt(deg[nz]).astype(np.float32)

    # Sort by src within each (dst-block, lo/hi) category: gather descriptors
    # then read ascending HBM addresses (row locality) instead of random.
    order = np.lexsort((src, c["hi"], c["gblk"]))
    src, dst, ew = src[order], dst[order], ew[order]
    gblk, hi, key = c["gblk"][order], c["hi"][order], c["key"][order]

    nlo = nlo.copy()
    none = (nlo + nhi) == 0
    nlo[none] = 1
    lay = _group_layout(nlo, nhi)
    totch = lay["totch"]

    nkeys = N_CORES * nblk * 2
    st = np.zeros(nkeys + 1, np.int64)
    st[1:] = np.cumsum(c["cnt"])
    pos = np.arange(E, dtype=np.int64) - st[key]

    core = gblk // nblk
    b = gblk % nblk
    chunk = np.where(hi == 1, lay["hi_col"][b], lay["lo_col"][b]) + pos // P
    # Within a chunk, map sorted-edge rank q to partition 16*(q%8) + q//8 so
    # SDMA engine b (= partition % 16) drains 8 consecutive sorted edges
    # (2KB of nearly-contiguous HBM reads) per chunk.
    q = pos % P
    part = 16 * (q % 8) + q // 8
    val = (src - hi * LO).astype(np.int16)

    idxv = np.zeros((N_CORES, P, totch), np.int16)
    dstl = np.zeros((N_CORES, P, totch), np.float32)
    nrmv = np.zeros((N_CORES, P, totch), np.float32)
    idxv[core, part, chunk] = val
    dstl[core, part, chunk] = (dst % P).astype(np.float32)
    nrmv[core, part, chunk] = ew.astype(np.float32)

    per_core = []
    for cc in range(N_CORES):
        A = np.ascontiguousarray(idxv[cc].T).reshape(totch, 8, 16)
        idx_w = np.tile(
            np.ascontiguousarray(A.transpose(2, 0, 1)).reshape(16, totch * 8), (8, 1))
        per_core.append(dict(idx=np.ascontiguousarray(idx_w),
                             dstloc=np.ascontiguousarray(dstl[cc]),
                             normv=np.ascontiguousarray(nrmv[cc])))
    return dict(per_core=per_core, nlo=nlo, nhi=nhi, totch=totch, lay=lay,
                dinv=dinv, recip=recip, nblk=nblk, sh=sh)


def build_stack_program(sh, nblk, totch, nlo, nhi, lay, npad, reps=1):
    import concourse.bacc as bacc
    import concourse.mybir as mybir
    import concourse.tile as tile

    f32 = mybir.dt.float32
    f16 = mybir.dt.float16
    i16 = mybir.dt.int16

    nc = bacc.Bacc("TRN2", target_bir_lowering=False, num_swdge_queues=4)
    x_in = nc.declare_dram_parameter("x", [sh, FC], f16, isOutput=False)
    w_in = nc.declare_dram_parameter("w", [3, FC, FC], f16, isOutput=False)
    b_in = nc.declare_dram_parameter("b", [3, FC], f16, isOutput=False)
    iota_in = nc.declare_dram_parameter("iota", [P, P], f16, isOutput=False)
    dinv_in = nc.declare_dram_parameter("dinv_cols", [P, nblk], f32, isOutput=False)
    recip_in = nc.declare_dram_parameter("recip_row", [1, sh], f16, isOutput=False)
    idx_in = nc.declare_dram_parameter("idx", [P, totch * 8], i16, isOutput=False)
    dst_in = nc.declare_dram_parameter("dstloc", [P, totch], f32, isOutput=False)
    nrm_in = nc.declare_dram_parameter("normv", [P, totch], f32, isOutput=False)
    outs = [nc.declare_dram_parameter(f"out{l}", [sh, FC], f16, isOutput=True)
            for l in range(3)]
    douts = [[nc.dram_tensor(f"dout{r}_{l}", [sh, FC], f16) for l in range(3)]
             for r in range(reps - 1)]
    h_shards = [nc.dram_tensor(f"h_shard{l}", [sh, FC], f16) for l in range(3 * reps)]
    h_fulls = [nc.dram_tensor(f"h_full{l}", [npad, FC], f16, addr_space="Shared")
               for l in range(3 * reps)]

    ngrp = lay["ngrp"]
    glo, ghi, gstart = lay["glo"], lay["ghi"], lay["gstart"]
    lo_col, hi_col = lay["lo_col"], lay["hi_col"]

    with tile.TileContext(nc) as tc:
        with (
            tc.tile_pool(name="sbuf", bufs=4) as pool,
            tc.tile_pool(name="psum", bufs=2, space="PSUM") as psum_pool,
            tc.tile_pool(name="psum_s", bufs=6, space="PSUM") as psum_spool,
            tc.tile_pool(name="gpool", bufs=6) as gpool,
            tc.tile_pool(name="mpool", bufs=32) as mpool,
            tc.tile_pool(name="const", bufs=1) as cpool,
        ):
            iota_t = cpool.tile([P, P], f16)
            nc.sync.dma_start(out=iota_t[:], in_=iota_in[:])
            w_ts, b_ts = [], []
            for l in range(3):
                w_t = cpool.tile([FC, FC], f16, tag=f"w{l}")
                nc.sync.dma_start(out=w_t[:], in_=w_in[l])
                w_ts.append(w_t)
                b_t = cpool.tile([1, FC], f16, tag=f"b{l}")
                nc.sync.dma_start(out=b_t[:], in_=b_in[l:l + 1, :])
                b_ts.append(b_t)
            dinv_t = cpool.tile([P, nblk], f32)
            nc.sync.dma_start(out=dinv_t[:], in_=dinv_in[:])
            recip_t = cpool.tile([1, sh], f16)
            nc.sync.dma_start(out=recip_t[:], in_=recip_in[:])
            idx_t = cpool.tile([P, totch * 8], i16)
            nc.sync.dma_start(out=idx_t[:], in_=idx_in[:])
            meta_d = cpool.tile([P, totch], f32)
            nc.sync.dma_start(out=meta_d[:], in_=dst_in[:])
            meta_n = cpool.tile([P, totch], f32)
            nc.sync.dma_start(out=meta_n[:], in_=nrm_in[:])

            for rep in range(reps):
              router = outs if rep == reps - 1 else douts[rep]
              for l in range(3):
                x_src = x_in if l == 0 else router[l - 1]
                h_shard, h_full = h_shards[rep * 3 + l], h_fulls[rep * 3 + l]
                for rt in range(0, nblk, TRW):
                    tw = min(TRW, nblk - rt)
                    xt_t = pool.tile([P, TRW * P], f16, tag="xt_t")
                    nc.sync.dma_start(out=xt_t[:, 0:tw * P],
                                      in_=x_src[rt * P:(rt + tw) * P, :],
                                      transpose=True)
                    h_w = pool.tile([P, TRW, P], f16, tag="h_w")
                    for j in range(tw):
                        r = rt + j
                        h_ps = psum_pool.tile([P, P], f32, space="PSUM", tag="h_ps")
                        nc.tensor.matmul(out=h_ps[:],
                                         lhsT=xt_t[:, j * P:(j + 1) * P],
                                         rhs=w_ts[l][:], start=True, stop=True)
                        nc.scalar.activation(out=h_w[:, j, :], in_=h_ps[:],
                                             func=mybir.ActivationFunctionType.Copy,
                                             scale=dinv_t[:, r:r + 1])
                    nc.sync.dma_start(
                        out=h_shard[:].rearrange("(b p) f -> p b f", p=P)[:, rt:rt + tw, :],
                        in_=h_w[:, 0:tw, :])
                nc.gpsimd.collective_compute(
                    "AllGather", mybir.AluOpType.bypass,
                    replica_groups=[list(range(N_CORES))],
                    ins=[h_shard[:]], outs=[h_full[:]])
                lo_end = min(LO, npad)
                h_lo = h_full[0:lo_end, :]
                h_hi = h_full[LO:npad, :] if npad > LO else None
                qctr = 0
                for gi in range(ngrp):
                    b0 = gi * GB
                    gw = min(GB, nblk - b0)
                    gl, gh, g0 = int(glo[gi]), int(ghi[gi]), int(gstart[gi])
                    gtot = gl + gh
                    g_t = gpool.tile([P, gtot, P], f16, tag="g_t")
                    if gl:
                        nc.gpsimd.dma_gather(
                            g_t[:, 0:gl, :], h_lo,
                            idx_t[:, g0 * 8:(g0 + gl) * 8], gl * P, gl * P, P,
                            single_packet=False, queue_num=qctr % 4)
                        qctr += 1
                    if gh:
                        nc.gpsimd.dma_gather(
                            g_t[:, gl:gtot, :], h_hi,
                            idx_t[:, (g0 + gl) * 8:(g0 + gtot) * 8],
                            gh * P, gh * P, P, single_packet=False,
                            queue_num=qctr % 4)
                        qctr += 1
                    o_w = pool.tile([P, GB, P], f16, tag="o_w")
                    for j in range(gw):
                        bk = b0 + j
                        nl, nh = int(nlo[bk]), int(nhi[bk])
                        ps = psum_spool.tile([P, P], f32, space="PSUM", tag="spmm_ps")
                        first = True
                        for noff, nch in ((int(lo_col[bk]) - g0, nl),
                                          (int(hi_col[bk]) - g0, nh)):
                            for cc in range(nch):
                                col = g0 + noff + cc
                                m_t = mpool.tile([P, P], f16, tag="m_t")
                                nc.vector.tensor_scalar(
                                    out=m_t[:], in0=iota_t[:],
                                    scalar1=meta_d[:, col:col + 1],
                                    scalar2=meta_n[:, col:col + 1],
                                    op0=mybir.AluOpType.is_equal,
                                    op1=mybir.AluOpType.mult)
                                nc.tensor.matmul(out=ps[:], lhsT=m_t[:],
                                                 rhs=g_t[:, noff + cc, :],
                                                 start=first, stop=False)
                                first = False
                        nc.tensor.matmul(out=ps[:],
                                         lhsT=recip_t[:, bk * P:(bk + 1) * P],
                                         rhs=b_ts[l][:], start=first, stop=True)
                        nc.scalar.activation(
                            out=o_w[:, j, :], in_=ps[:],
                            func=mybir.ActivationFunctionType.Relu,
                            scale=dinv_t[:, bk:bk + 1])
                    nc.sync.dma_start(
                        out=router[l][:].rearrange("(b p) f -> p b f", p=P)[:, b0:b0 + gw, :],
                        in_=o_w[:, 0:gw, :])
    nc.finalize()
    return nc


_iota_np = np.tile(np.arange(P, dtype=np.float16)[None, :], (P, 1))


def make_in_maps(g, x0_full_f16, W3, b3):
    sh, nblk = g["sh"], g["nblk"]
    in_maps = []
    for c in range(N_CORES):
        pc = g["per_core"][c]
        in_maps.append(dict(
            x=np.ascontiguousarray(x0_full_f16[c * sh:(c + 1) * sh]),
            w=W3.astype(np.float16), b=b3.astype(np.float16), iota=_iota_np,
            dinv_cols=np.ascontiguousarray(
                g["dinv"][c * sh:(c + 1) * sh].reshape(nblk, P).T),
            recip_row=g["recip"][c * sh:(c + 1) * sh][None, :].astype(np.float16),
            idx=pc["idx"], dstloc=pc["dstloc"], normv=pc["normv"],
        ))
    return in_maps


def run_stack(nc, g, x0_full_f16, W3, b3, npad):
    from concourse.bass_utils import run_bass_kernel_spmd
    in_maps = make_in_maps(g, x0_full_f16, W3, b3)
    res = run_bass_kernel_spmd(nc, in_maps, core_ids=list(range(N_CORES)))
    out = np.zeros((3, npad, FC), np.float32)
    for l in range(3):
        out[l] = np.concatenate(
            [res.results[c][f"out{l}"].astype(np.float32) for c in range(N_CORES)],
            axis=0)
    return out


def prep_family(edges_list, w_list, n_real, npad):
    """Prep the 3 graphs of one family with unified chunk counts so a single
    compiled program serves all three."""
    counts = [_prep_counts(e[0], e[1], w, n_real, npad)
              for e, w in zip(edges_list, w_list)]
    nlo = np.maximum.reduce([c["nlo"] for c in counts])
    nhi = np.maximum.reduce([c["nhi"] for c in counts])
    return [_prep_pack(c, npad, nlo, nhi) for c in counts]


def _pca_host(Xm, k):
    mu = Xm.mean(axis=0, keepdims=True, dtype=np.float32)
    Xc = (Xm - mu).astype(np.float32)
    cov = (Xc.T @ Xc) / np.float32(Xm.shape[0] - 1)
    try:
        import jax
        with jax.default_device(jax.devices("cpu")[0]):
            _, V = jax.numpy.linalg.eigh(cov)
            V = np.asarray(V)
    except Exception:
        _, V = np.linalg.eigh(cov)
    comp = V[:, ::-1][:, :k]
    return Xc @ comp.astype(np.float32)


def _l2norm(x):
    n = np.sqrt((x.astype(np.float32) ** 2).sum(axis=1, keepdims=True))
    return x / np.maximum(n, 1e-12)


def kernel(**inputs):
    inp = {k: np.asarray(v) for k, v in inputs.items()}
    x_fc = inp["x_fc"].astype(np.float32)
    y_fd = inp["y_fd"].astype(np.float32)
    Wc, bc = inp["Wc"].astype(np.float32), inp["bc"].astype(np.float32)
    Wd, bd = inp["Wd"].astype(np.float32), inp["bd"].astype(np.float32)

    x0c = np.zeros((NPAD_C, FC), np.float16)
    x0c[:NC] = x_fc.astype(np.float16)
    x0d = np.zeros((NPAD_D, FC), np.float16)
    x0d[:ND] = y_fd.astype(np.float16)

    cc_names = ["cc_g", "cc_c", "cc_sem"]
    dd_names = ["dd_g", "dd_c", "dd_dag"]
    gs_cc = prep_family([inp[f"{n}_edges"] for n in cc_names],
                        [inp[f"{n}_w"] for n in cc_names], NC, NPAD_C)
    gs_dd = prep_family([inp[f"{n}_edges"] for n in dd_names],
                        [inp[f"{n}_w"] for n in dd_names], ND, NPAD_D)

    g0 = gs_cc[0]
    nc_cc = build_stack_program(g0["sh"], g0["nblk"], g0["totch"],
                                g0["nlo"], g0["nhi"], g0["lay"], NPAD_C)
    g0 = gs_dd[0]
    nc_dd = build_stack_program(g0["sh"], g0["nblk"], g0["totch"],
                                g0["nlo"], g0["nhi"], g0["lay"], NPAD_D)

    xs_out = []
    for i, g in enumerate(gs_cc):
        o = run_stack(nc_cc, g, x0c, Wc[3 * i:3 * i + 3], bc[3 * i:3 * i + 3],
                      NPAD_C)
        xs_out.extend([o[0][:NC], o[1][:NC], o[2][:NC]])
    ys_out = []
    for i, g in enumerate(gs_dd):
        o = run_stack(nc_dd, g, x0d, Wd[3 * i:3 * i + 3], bd[3 * i:3 * i + 3],
                      NPAD_D)
        ys_out.extend([o[0][:ND], o[1][:ND], o[2][:ND]])

    XM = np.concatenate(xs_out, axis=1)
    YD = np.concatenate(ys_out, axis=1)
    XM = _l2norm(_pca_host(XM, K))
    YD = _l2norm(_pca_host(YD, K))
    XM = np.concatenate([XM, inp["Gra_emb_circrna"].astype(np.float32)], axis=1)
    YD = np.concatenate([YD, inp["Gra_emb_dis"].astype(np.float32)], axis=1)

    ci = inp["circ_index"].astype(np.int64)
    di = inp["dis_index"].astype(np.int64)
    c = XM[ci]
    d = YD[di]
    dec_W = inp["dec_W"].astype(np.float32)
    dec_cls = inp["dec_cls"].astype(np.float32)
    basis = np.stack([((c @ dec_W[k_]) * d).sum(axis=1) for k_ in range(2)], axis=1)
    return np.maximum(basis @ dec_cls, 0.0).astype(np.float32)

